# revision 2
# baseline (speedup 1.0000x reference)
"""LConv (7x7 position-linear conv) Trainium2 Bass kernel.

Full inputs in, full output out. Sharding: data-parallel over batch,
16 images -> 8 NeuronCores (2 images/core). abc/bias replicated.

Math (K=7, S=1, P=3, D=1, G=1, C=COUT=128):
  out[o,i,j] = sum_c sum_{t=1..7} P_t[c,o]*W1[c,i+t,j] + B[c,o]*W2[c,i+t,j]
             + bias[o]
  W1 = 7-wide box sum along W of padded x; W2 = position-ramp sum along W.
  Both are computed from running sums (cumsum) along each padded row:
  rows are stored 122 wide = 7 lead zeros + 112 data + 3 trail zeros, so
  cumsum(stream shifted by 7) - cumsum(stream) telescopes exactly to the
  7-tap box (the 7 lead zeros kill the stream-prefix offset).
  P_t = (t-4)*A + C ; A=abc[0:128], B=abc[128:256], C=abc[256:384].
"""

import numpy as np

import concourse.bacc as bacc
import concourse.mybir as mybir
from concourse import tile
from concourse.bass_utils import run_bass_kernel_spmd

F32 = mybir.dt.float32
BF16 = mybir.dt.bfloat16
I32 = mybir.dt.int32
AF = mybir.ActivationFunctionType
ALU = mybir.AluOpType

USE_CUSTOM_OPA = True   # fused scan-diff custom DVE op
USE_PAIR_SUMS = True    # gpsimd pair-sums -> 11 matmuls/tile instead of 14

B_TOT = 16
N_CORES = 8
B_PER = B_TOT // N_CORES
CIN = 128
COUT = 128
H = W = 112
PW2 = 122                 # row layout: 7 lead zeros + 112 data + 3 trail
LEAD = 7
PH = 119                  # padded row count: 4 lead + 112 + 3 trail
RLEAD = 4
ROWS_PER_SLAB = 16
N_SLABS = H // ROWS_PER_SLAB
SLAB_FREE = ROWS_PER_SLAB * PW2        # 1952
DSTREAM = SLAB_FREE - 7                # 1945
WFULL = PH * PW2                       # 14518 (stream layout, bf16)
OUT_TILE_ROWS = 4
N_OUT_TILES = H // OUT_TILE_ROWS
OUT_TILE_FREE = OUT_TILE_ROWS * W      # 448

_CACHE = {}


def _register_opa():
    from concourse.dve_spec import Spec, Src0, Src1, scan, AluOp, lower
    import concourse.dve_ops as dve_ops
    from concourse.dve_uop import DveOpSpec

    if any(op.name == "BOXDIFF7" for op in dve_ops.OPS):
        return next(op for op in dve_ops.OPS if op.name == "BOXDIFF7")
    spec = Spec(
        body=scan(AluOp.ADD, Src0) - scan(AluOp.ADD, Src1),
        reference=lambda in0, in1: (
            np.cumsum(in0, axis=-1) - np.cumsum(in1, axis=-1)
        ),
    )
    row = dve_ops._CUSTOM_DVE_ROW_BASE + len(dve_ops.OPS)
    shas = {}
    for ver in ("v3", "v4"):
        s = DveOpSpec(
            name="BOXDIFF7", opcode=row, uops=lower(spec, ver=ver), rd1_en=True
        )
        shas[ver] = s.sha(ver)
    op = dve_ops.DveOp("BOXDIFF7", spec, subdim=False, uops_sha=shas)
    dve_ops.OPS.append(op)
    dve_ops._SUB_OPCODE_FOR_NAME[op.name] = row
    dve_ops.CUSTOM_DVE_SPECS[op.name] = op.spec
    return op


def _build():
    nc = bacc.Bacc("TRN2", target_bir_lowering=False, debug=False)
    opa = _register_opa() if USE_CUSTOM_OPA else None

    t_x = nc.dram_tensor("xs", [B_PER, CIN, H, W], F32, kind="ExternalInput")
    t_pw = nc.dram_tensor("pw", [7, CIN, COUT], F32, kind="ExternalInput")
    t_bw = nc.dram_tensor("bw", [CIN, COUT], F32, kind="ExternalInput")
    t_bias = nc.dram_tensor("bias", [COUT, 1], F32, kind="ExternalInput")
    t_out = nc.dram_tensor("out", [B_PER, COUT, H, W], F32, kind="ExternalOutput")

    with tile.TileContext(nc) as tc:
        with (
            tc.tile_pool(name="const", bufs=1) as cpool,
            tc.tile_pool(name="wfull", bufs=1) as wpool,
            tc.tile_pool(name="slab", bufs=2) as spool,
            tc.tile_pool(name="outs", bufs=3) as opool,
            tc.tile_pool(name="ps", bufs=4, space="PSUM") as ppool,
        ):
            # ---- constants ----
            pw_f = cpool.tile([CIN, 7 * COUT], F32, tag="pwf")
            nc.sync.dma_start(
                pw_f[:].rearrange("c (t o) -> c t o", t=7),
                t_pw[:].transpose([1, 0, 2]),
            )
            pw = cpool.tile([CIN, 7 * COUT], BF16, tag="pwb")
            nc.vector.tensor_copy(pw[:], pw_f[:])
            bw_f = cpool.tile([CIN, COUT], F32, tag="bwf")
            nc.sync.dma_start(bw_f[:], t_bw[:])
            bw = cpool.tile([CIN, COUT], BF16, tag="bwb")
            nc.vector.tensor_copy(bw[:], bw_f[:])
            bias_sb = cpool.tile([COUT, 1], F32, tag="bias")
            nc.sync.dma_start(bias_sb[:], t_bias[:])

            # col-position map for the ramp: value (p-3) at col p of the
            # 122-grid == (data col + 4), matching the recenter term (j+4).
            jmap_i = cpool.tile([128, SLAB_FREE], I32, tag="jmapi")
            nc.gpsimd.iota(
                jmap_i[:], pattern=[[0, ROWS_PER_SLAB], [1, PW2]],
                base=-3, channel_multiplier=0,
            )
            jmap = cpool.tile([128, SLAB_FREE], F32, tag="jmap")
            nc.vector.tensor_copy(jmap[:], jmap_i[:])
            jp4_i = cpool.tile([128, ROWS_PER_SLAB * W], I32, tag="jp4i")
            nc.gpsimd.iota(
                jp4_i[:], pattern=[[0, ROWS_PER_SLAB], [1, W]],
                base=4, channel_multiplier=0,
            )
            jp4 = cpool.tile([128, ROWS_PER_SLAB * W], F32, tag="jp4")
            nc.vector.tensor_copy(jp4[:], jp4_i[:])

            # ---- full-image W1/W2 streams (bf16, PH x PW2 layout) ----
            w1 = wpool.tile([CIN, WFULL], BF16, tag="w1")
            w2 = wpool.tile([CIN, WFULL], BF16, tag="w2")
            nc.vector.memset(w1[:, : RLEAD * PW2], 0.0)
            nc.vector.memset(w1[:, (RLEAD + H) * PW2 :], 0.0)
            nc.vector.memset(w2[:, : RLEAD * PW2], 0.0)
            nc.vector.memset(w2[:, (RLEAD + H) * PW2 :], 0.0)
            if USE_PAIR_SUMS:
                w2p = wpool.tile([CIN, WFULL], BF16, tag="w2p")
                nc.vector.memset(w2p[:, : RLEAD * PW2], 0.0)
                nc.vector.memset(w2p[:, (RLEAD + H - 1) * PW2 :], 0.0)

            xp_bufs = []
            for i in range(2):
                xpb = spool.tile([CIN, SLAB_FREE], F32, tag=f"xp{i}")
                nc.vector.memset(xpb[:], 0.0)
                xp_bufs.append(xpb)

            def row_view(buf, r0, nrows=ROWS_PER_SLAB):
                # strided (nrows,112) view at data cols of the 122-grid
                base = (RLEAD + r0) * PW2
                return buf[:, base : base + nrows * PW2].rearrange(
                    "c (r q) -> c r q", q=PW2
                )[:, :, 3:115]

            for b in range(B_PER):
                # ---------- stage 1: W-direction filters ----------
                for s in range(N_SLABS):
                    r0 = s * ROWS_PER_SLAB
                    xp = xp_bufs[s % 2]
                    nc.sync.dma_start(
                        xp[:].rearrange("c (r q) -> c r q", r=ROWS_PER_SLAB)[
                            :, :, LEAD : LEAD + W
                        ],
                        t_x[b, :, r0 : r0 + ROWS_PER_SLAB, :],
                    )
                    w1s = w1[:, (RLEAD + r0) * PW2 : (RLEAD + r0) * PW2 + DSTREAM]
                    w2s = w2[:, (RLEAD + r0) * PW2 : (RLEAD + r0) * PW2 + DSTREAM]
                    if USE_CUSTOM_OPA:
                        d1 = spool.tile([CIN, SLAB_FREE], F32, tag="d1")
                        nc.vector._custom_dve(
                            opa, out=d1[:, :DSTREAM], in0=xp[:, 7:], in1=xp[:, :DSTREAM]
                        )
                        nc.scalar.copy(w1s, d1[:, :DSTREAM])
                        nc.vector.tensor_tensor(xp[:], xp[:], jmap[:], op=ALU.mult)
                        rawd = spool.tile([CIN, SLAB_FREE], F32, tag="rawd")
                        nc.vector._custom_dve(
                            opa, out=rawd[:, :DSTREAM], in0=xp[:, 7:], in1=xp[:, :DSTREAM]
                        )
                    else:
                        c1 = spool.tile([CIN, SLAB_FREE], F32, tag="c1")
                        nc.vector.tensor_tensor_scan(
                            c1[:], xp[:], xp[:], 0.0, op0=ALU.add, op1=ALU.bypass
                        )
                        d1 = spool.tile([CIN, SLAB_FREE], F32, tag="d1")
                        nc.vector.tensor_tensor(
                            d1[:, :DSTREAM], c1[:, 7:], c1[:, :DSTREAM], op=ALU.subtract
                        )
                        nc.scalar.copy(w1s, d1[:, :DSTREAM])
                        nc.vector.tensor_tensor(xp[:], xp[:], jmap[:], op=ALU.mult)
                        cj = spool.tile([CIN, SLAB_FREE], F32, tag="cj")
                        nc.vector.tensor_tensor_scan(
                            cj[:], xp[:], xp[:], 0.0, op0=ALU.add, op1=ALU.bypass
                        )
                        rawd = spool.tile([CIN, SLAB_FREE], F32, tag="rawd")
                        nc.vector.tensor_tensor(
                            rawd[:, :DSTREAM], cj[:, 7:], cj[:, :DSTREAM], op=ALU.subtract
                        )
                    # w2b = (j+4) * W1  (gpsimd, fp32, strided d1 view)
                    d1v = d1[:].rearrange("c (r q) -> c r q", q=PW2)[:, :, 3:115]
                    w2b = spool.tile([CIN, ROWS_PER_SLAB * W], F32, tag="w2b")
                    nc.gpsimd.tensor_tensor(
                        w2b[:].rearrange("c (r j) -> c r j", r=ROWS_PER_SLAB),
                        jp4[:].rearrange("c (r j) -> c r j", r=ROWS_PER_SLAB),
                        d1v,
                        op=ALU.mult,
                    )
                    # w2 = rawd - w2b (bf16 cast on write; values are small)
                    rawv = rawd[:].rearrange("c (r q) -> c r q", q=PW2)[:, :, 3:115]
                    nc.vector.tensor_tensor(
                        row_view(w2, r0),
                        rawv,
                        w2b[:].rearrange("c (r j) -> c r j", r=ROWS_PER_SLAB),
                        op=ALU.subtract,
                    )
                    if USE_PAIR_SUMS and s > 0:
                        # pair rows r-1..: w2p[r] = w2[r] + w2[r+1]
                        pr0 = (s - 1) * ROWS_PER_SLAB
                        nc.gpsimd.tensor_tensor(
                            row_view(w2p, pr0),
                            row_view(w2, pr0),
                            row_view(w2, pr0 + 1),
                            op=ALU.add,
                        )
                if USE_PAIR_SUMS:
                    # last slab's pairs + the pad-row boundary pairs
                    pr0 = (N_SLABS - 1) * ROWS_PER_SLAB
                    nc.gpsimd.tensor_tensor(
                        row_view(w2p, pr0, ROWS_PER_SLAB - 1),
                        row_view(w2, pr0, ROWS_PER_SLAB - 1),
                        row_view(w2, pr0 + 1, ROWS_PER_SLAB - 1),
                        op=ALU.add,
                    )
                    # rows -4..-1 (lead pad rows -4..-2 pair into data row 0)
                    nc.gpsimd.tensor_tensor(
                        row_view(w2p, -4, 4),
                        row_view(w2, -4, 4),
                        row_view(w2, -3, 4),
                        op=ALU.add,
                    )
                    # trailing: row H-1 pairs with pad row H (zero) etc
                    nc.gpsimd.tensor_tensor(
                        row_view(w2p, H - 1, 3),
                        row_view(w2, H - 1, 3),
                        row_view(w2, H, 3),
                        op=ALU.add,
                    )

                # ---------- stage 2: PE folds over H-shifts ----------
                for it in range(N_OUT_TILES):
                    i0 = it * OUT_TILE_ROWS
                    acc = ppool.tile([COUT, OUT_TILE_FREE], F32, tag="acc")

                    def rhs(buf, trow):
                        base = (i0 + trow) * PW2
                        return buf[:, base : base + OUT_TILE_ROWS * PW2].rearrange(
                            "c (r q) -> c r q", q=PW2
                        )[:, :, 3:115]

                    first = True
                    for t in range(1, 8):
                        nc.tensor.matmul(
                            acc[:],
                            pw[:, (t - 1) * COUT : t * COUT],
                            rhs(w1, t),
                            start=first,
                            stop=False,
                        )
                        first = False
                    if USE_PAIR_SUMS:
                        # box7(w2) = w2p[1] + w2p[3] + w2p[5] + w2[7]
                        for t in (1, 3, 5):
                            nc.tensor.matmul(
                                acc[:], bw[:], rhs(w2p, t), start=False, stop=False
                            )
                        nc.tensor.matmul(
                            acc[:], bw[:], rhs(w2, 7), start=False, stop=True
                        )
                    else:
                        for t in range(1, 8):
                            nc.tensor.matmul(
                                acc[:], bw[:], rhs(w2, t), start=False, stop=(t == 7)
                            )
                    ot = opool.tile([COUT, OUT_TILE_FREE], F32, tag="ot")
                    nc.scalar.activation(
                        ot[:], acc[:], AF.Identity, bias=bias_sb[:], scale=1.0
                    )
                    nc.sync.dma_start(
                        t_out[b, :, i0 : i0 + OUT_TILE_ROWS, :].rearrange(
                            "o r j -> o (r j)"
                        ),
                        ot[:],
                    )

    nc.compile()
    return nc


def make_in_maps(x, abc, bias):
    x = np.ascontiguousarray(x, dtype=np.float32)
    abc = np.asarray(abc, dtype=np.float32)
    bias = np.asarray(bias, dtype=np.float32)
    A, Bm, Cc = abc[0:128], abc[128:256], abc[256:384]
    pw = np.stack([(t - 4.0) * A + Cc for t in range(1, 8)]).astype(np.float32)
    in_maps = []
    for c in range(N_CORES):
        in_maps.append(
            {
                "xs": x[c * B_PER : (c + 1) * B_PER],
                "pw": pw,
                "bw": np.ascontiguousarray(Bm),
                "bias": np.ascontiguousarray(bias.reshape(COUT, 1)),
            }
        )
    return in_maps, N_CORES


def kernel(x: np.ndarray, abc: np.ndarray, bias: np.ndarray) -> np.ndarray:
    if "nc" not in _CACHE:
        _CACHE["nc"] = _build()
    nc = _CACHE["nc"]

    in_maps, _ = make_in_maps(x, abc, bias)
    res = run_bass_kernel_spmd(nc, in_maps, list(range(N_CORES)))
    out = np.concatenate([res.results[c]["out"] for c in range(N_CORES)], axis=0)
    return out.astype(np.float32)


if __name__ == "__main__":
    rng = np.random.default_rng(0)
    x = rng.standard_normal((16, 128, 112, 112), dtype=np.float32)
    abc = (rng.standard_normal((384, 128)) * 0.05).astype(np.float32)
    bias = (rng.standard_normal((128,)) * 0.05).astype(np.float32)
    out = kernel(x=x, abc=abc, bias=bias)
    print(out.shape, out.dtype)



# revision 4
# speedup vs baseline: 1.5137x; 1.5137x over previous
"""LConv (7x7 position-linear conv) Trainium2 Bass kernel.

Full inputs in, full output out. Sharding: data-parallel over batch,
16 images -> 8 NeuronCores (2 images/core). abc/bias replicated.

Math (K=7, S=1, P=3, D=1, G=1, C=COUT=128):
  out[o,i,j] = sum_c sum_{t=1..7} P_t[c,o]*W1[c,i+t,j] + B[c,o]*W2[c,i+t,j]
             + bias[o]
  W1 = 7-wide box sum along W of padded x; W2 = centered 7-tap ramp
  (sum_t (t-4) x[j+t]) along W. Rows are stored 122 wide = 7 lead zeros +
  112 data + 3 trail zeros, so with the stream cumsums S0=cumsum(x),
  S1=cumsum(x[+7]):
     W1 = S1 - S0                      (telescoped box)
     W2 = 7*S0 + 4*(S1-S0) - cumsum(S1-S0)   (telescoped ramp; exact)
  Both are one fused DVE pass each (BOXDIFF7 / hand-authored RAMPW7).
  P_t = (t-4)*A + C ; A=abc[0:128], B=abc[128:256], C=abc[256:384].
"""

import numpy as np

import concourse.bacc as bacc
import concourse.mybir as mybir
from concourse import tile
from concourse.bass_utils import run_bass_kernel_spmd

F32 = mybir.dt.float32
BF16 = mybir.dt.bfloat16
I32 = mybir.dt.int32
AF = mybir.ActivationFunctionType
ALU = mybir.AluOpType

USE_PAIR_SUMS = True    # gpsimd pair-sums -> 11 matmuls/tile instead of 14

B_TOT = 16
N_CORES = 8
B_PER = B_TOT // N_CORES
CIN = 128
COUT = 128
H = W = 112
PW2 = 122                 # row layout: 7 lead zeros + 112 data + 3 trail
LEAD = 7
PH = 119                  # padded row count: 4 lead + 112 + 3 trail
RLEAD = 4
ROWS_PER_SLAB = 16
N_SLABS = H // ROWS_PER_SLAB
SLAB_FREE = ROWS_PER_SLAB * PW2        # 1952
DSTREAM = SLAB_FREE - 7                # 1945
WFULL = PH * PW2                       # 14518 (stream layout, bf16)
OUT_TILE_ROWS = 4
N_OUT_TILES = H // OUT_TILE_ROWS
OUT_TILE_FREE = OUT_TILE_ROWS * W      # 448

_CACHE = {}


def _register_boxdiff7():
    """out = cumsum(in0) - cumsum(in1)  (the telescoped 7-tap box)."""
    from concourse.dve_spec import Spec, Src0, Src1, scan, AluOp, lower
    import concourse.dve_ops as dve_ops
    from concourse.dve_uop import DveOpSpec

    if any(op.name == "BOXDIFF7" for op in dve_ops.OPS):
        return next(op for op in dve_ops.OPS if op.name == "BOXDIFF7")
    spec = Spec(
        body=scan(AluOp.ADD, Src0) - scan(AluOp.ADD, Src1),
        reference=lambda in0, in1, *a: (
            np.cumsum(in0, axis=-1) - np.cumsum(in1, axis=-1)
        ),
    )
    row = dve_ops._CUSTOM_DVE_ROW_BASE + len(dve_ops.OPS)
    shas = {}
    for ver in ("v3", "v4"):
        s = DveOpSpec(
            name="BOXDIFF7", opcode=row, uops=lower(spec, ver=ver), rd1_en=True
        )
        shas[ver] = s.sha(ver)
    op = dve_ops.DveOp("BOXDIFF7", spec, subdim=False, uops_sha=shas)
    dve_ops.OPS.append(op)
    dve_ops._SUB_OPCODE_FOR_NAME[op.name] = row
    dve_ops.CUSTOM_DVE_SPECS[op.name] = op.spec
    return op


class _HandDveOp:
    """Duck-typed DveOp whose uop program is hand-authored (the Spec DSL
    cannot express a scan-of-scan)."""

    def __init__(self, name, spec, subdim, build):
        self.name = name
        self.spec = spec
        self.subdim = subdim
        self._build = build
        self._cache = {}

    def compile(self, ver):
        if ver not in self._cache:
            self._cache[ver] = self._build(self.name, ver)
        return self._cache[ver]


def _rampw7_ref(in0, in1, *a):
    s0 = np.cumsum(in0, axis=-1)
    s1 = np.cumsum(in1, axis=-1)
    d = s1 - s0
    return 7.0 * s0 + 4.0 * d - np.cumsum(d, axis=-1)


def _build_rampw7_uops(name, ver):
    """out = 7*S0 + 4*D - E with S0=scan(src0), S1=scan(src1), D=S1-S0,
    E=scan(D). Called with src0=x, src1=x[+7] this is the centered 7-tap
    ramp filter sum_t (t-4)*x[k+t] (exact by Abel summation, given the 7
    lead zeros at stream start).

    Pipeline (8 blocks, 5 delay lanes with reuse):
      b0: S0 = ADD(CURR, lane0=src0)      ; b1 captures S0 -> lane0
      b1: S1 = ADD(CURR, lane1=src1)
      b2: D  = SUB(PREV, lane0=S0)
      b3: E  = ADD(CURR, PREV)            ; captures D -> lane1
      b4: M  = MUL(lane0=S0, lane2=7.0)   ; captures E -> lane2
      b5: N  = MUL(lane1=D, lane3=4.0)    ; captures M -> lane3
      b6: P  = ADD(PREV, lane3=M)
      b7: out= SUB(PREV, lane2=E)
    Seed uop zeroes the three scan accumulators (b0, b1, b3)."""
    from concourse.dve_uop import (
        DveOpSpec,
        UopConfig,
        UopDpConfig,
        AluOp,
        AluInp,
        InpSel,
        OutSel,
        OutPath,
        DelayInp,
        Trigger,
        ENABLE,
    )
    import concourse.dve_ops as dve_ops

    LANES = (0, 1, 2, 3, 4)
    IN_PREV = AluInp.PREV_ALU_OUT
    IN_CURR = AluInp.CURR_ALU_OUT

    def lane(n):
        return AluInp(int(AluInp.PREV_DELAY_0) + n)

    def wire_inputs(u):
        u.enable_input(InpSel.SRC_0, 1)    # lane 0
        u.enable_input(InpSel.SRC_1, 2)    # lane 1
        u.enable_input(InpSel.CONST_0, 3)  # lane 2 = s0 = 7.0
        u.enable_input(InpSel.CONST_1, 4)  # lane 3 = s1 = 4.0
        u.enable_input(InpSel.ZERO, 5)     # lane 4 = 0.0 (seed)
        return u

    def body_dp():
        dp = [UopDpConfig() for _ in range(8)]
        for st in range(8):
            dp[st].pass_through_delay(*LANES)
        dp[0].enable_alu(AluOp.ADD, IN_CURR, lane(0))
        dp[1].enable_alu(AluOp.ADD, IN_CURR, lane(1))
        dp[1].enable_delay_from_src(DelayInp.PREV_ALU_OUT, 0)
        dp[2].enable_alu(AluOp.SUBTRACT, IN_PREV, lane(0))
        dp[3].enable_alu(AluOp.ADD, IN_CURR, IN_PREV)
        dp[3].enable_delay_from_src(DelayInp.PREV_ALU_OUT, 1)
        dp[4].enable_alu(AluOp.MULTIPLY, lane(0), lane(2))
        dp[4].enable_delay_from_src(DelayInp.PREV_ALU_OUT, 2)
        dp[5].enable_alu(AluOp.MULTIPLY, lane(1), lane(3))
        dp[5].enable_delay_from_src(DelayInp.PREV_ALU_OUT, 3)
        dp[6].enable_alu(AluOp.ADD, IN_PREV, lane(3))
        dp[7].enable_alu(AluOp.SUBTRACT, IN_PREV, lane(2))
        return dp

    # seed: one COUNT cycle, overrides the scan blocks to output 0
    seed = wire_inputs(UopConfig())
    seed.datapath_config = body_dp()
    for b in (0, 1, 3):
        seed.datapath_config[b].enable_alu(AluOp.BYPASS, lane(4), lane(4))
    seed.trigger = (Trigger.COUNT, Trigger.NONE, Trigger.NONE)
    seed.repeat_count = 1
    seed.next_uop = (1, 0, 0)

    steady = wire_inputs(UopConfig())
    steady.datapath_config = body_dp()
    steady.trigger = (Trigger.SRC_TENSOR_DONE, Trigger.NONE, Trigger.NONE)
    steady.next_uop = (0, 0, 0)
    steady.require_inp0 = 1
    steady.require_inp1 = 1
    steady.enable_output(OutSel.ALU_OUT, OutPath.WR0_LO)

    spec = DveOpSpec(
        name=name,
        opcode=dve_ops.get_dve_sub_opcode(name),
        uops=[seed, steady],
        rd1_en=True,
    )
    spec.validate(ver)
    return spec


def _register_rampw7():
    from concourse.dve_spec import Spec, Src0, Src1, scan, AluOp
    import concourse.dve_ops as dve_ops

    if any(op.name == "RAMPW7" for op in dve_ops.OPS):
        return next(op for op in dve_ops.OPS if op.name == "RAMPW7")
    # spec is only used for leaf/accum introspection and the CoreSim
    # reference; the actual table program comes from _build_rampw7_uops.
    spec = Spec(
        body=scan(AluOp.ADD, Src0) - scan(AluOp.ADD, Src1),
        reference=_rampw7_ref,
    )
    row = dve_ops._CUSTOM_DVE_ROW_BASE + len(dve_ops.OPS)
    op = _HandDveOp("RAMPW7", spec, False, _build_rampw7_uops)
    dve_ops.OPS.append(op)
    dve_ops._SUB_OPCODE_FOR_NAME[op.name] = row
    dve_ops.CUSTOM_DVE_SPECS[op.name] = op.spec
    return op


def _build():
    nc = bacc.Bacc("TRN2", target_bir_lowering=False, debug=False)
    opa = _register_boxdiff7()
    opr = _register_rampw7()

    t_x = nc.dram_tensor("xs", [B_PER, CIN, H, W], F32, kind="ExternalInput")
    t_pw = nc.dram_tensor("pw", [7, CIN, COUT], F32, kind="ExternalInput")
    t_bw = nc.dram_tensor("bw", [CIN, COUT], F32, kind="ExternalInput")
    t_bias = nc.dram_tensor("bias", [COUT, 1], F32, kind="ExternalInput")
    t_out = nc.dram_tensor("out", [B_PER, COUT, H, W], F32, kind="ExternalOutput")

    with tile.TileContext(nc) as tc:
        with (
            tc.tile_pool(name="const", bufs=1) as cpool,
            tc.tile_pool(name="wfull", bufs=1) as wpool,
            tc.tile_pool(name="slab", bufs=2) as spool,
            tc.tile_pool(name="outs", bufs=3) as opool,
            tc.tile_pool(name="ps", bufs=4, space="PSUM") as ppool,
        ):
            # ---- constants ----
            pw_f = cpool.tile([CIN, 7 * COUT], F32, tag="pwf")
            nc.sync.dma_start(
                pw_f[:].rearrange("c (t o) -> c t o", t=7),
                t_pw[:].transpose([1, 0, 2]),
            )
            pw = cpool.tile([CIN, 7 * COUT], BF16, tag="pwb")
            nc.vector.tensor_copy(pw[:], pw_f[:])
            bw_f = cpool.tile([CIN, COUT], F32, tag="bwf")
            nc.sync.dma_start(bw_f[:], t_bw[:])
            bw = cpool.tile([CIN, COUT], BF16, tag="bwb")
            nc.vector.tensor_copy(bw[:], bw_f[:])
            bias_sb = cpool.tile([COUT, 1], F32, tag="bias")
            nc.sync.dma_start(bias_sb[:], t_bias[:])

            # ---- full-image W1/W2 streams (bf16, PH x PW2 layout) ----
            w1 = wpool.tile([CIN, WFULL], BF16, tag="w1")
            w2 = wpool.tile([CIN, WFULL], BF16, tag="w2")
            nc.vector.memset(w1[:, : RLEAD * PW2], 0.0)
            nc.vector.memset(w1[:, (RLEAD + H) * PW2 :], 0.0)
            nc.vector.memset(w2[:, : RLEAD * PW2], 0.0)
            nc.vector.memset(w2[:, (RLEAD + H) * PW2 :], 0.0)
            if USE_PAIR_SUMS:
                w2p = wpool.tile([CIN, WFULL], BF16, tag="w2p")
                nc.vector.memset(w2p[:, : RLEAD * PW2], 0.0)
                nc.vector.memset(w2p[:, (RLEAD + H - 1) * PW2 :], 0.0)

            xp_bufs = []
            for i in range(2):
                xpb = spool.tile([CIN, SLAB_FREE], F32, tag=f"xp{i}")
                nc.vector.memset(xpb[:], 0.0)
                xp_bufs.append(xpb)

            def row_view(buf, r0, nrows=ROWS_PER_SLAB):
                # strided (nrows,112) view at data cols of the 122-grid
                base = (RLEAD + r0) * PW2
                return buf[:, base : base + nrows * PW2].rearrange(
                    "c (r q) -> c r q", q=PW2
                )[:, :, 3:115]

            for b in range(B_PER):
                # ---------- stage 1: W-direction filters ----------
                for s in range(N_SLABS):
                    r0 = s * ROWS_PER_SLAB
                    xp = xp_bufs[s % 2]
                    nc.sync.dma_start(
                        xp[:].rearrange("c (r q) -> c r q", r=ROWS_PER_SLAB)[
                            :, :, LEAD : LEAD + W
                        ],
                        t_x[b, :, r0 : r0 + ROWS_PER_SLAB, :],
                    )
                    w1s = w1[:, (RLEAD + r0) * PW2 : (RLEAD + r0) * PW2 + DSTREAM]
                    w2s = w2[:, (RLEAD + r0) * PW2 : (RLEAD + r0) * PW2 + DSTREAM]
                    nc.vector._custom_dve(
                        opa, out=w1s, in0=xp[:, 7:], in1=xp[:, :DSTREAM]
                    )
                    nc.vector._custom_dve(
                        opr, out=w2s, in0=xp[:, :DSTREAM], in1=xp[:, 7:],
                        s0=7.0, s1=4.0,
                    )
                    if USE_PAIR_SUMS and s > 0:
                        # pair rows r-1..: w2p[r] = w2[r] + w2[r+1]
                        pr0 = (s - 1) * ROWS_PER_SLAB
                        nc.gpsimd.tensor_tensor(
                            row_view(w2p, pr0),
                            row_view(w2, pr0),
                            row_view(w2, pr0 + 1),
                            op=ALU.add,
                        )
                if USE_PAIR_SUMS:
                    # last slab's pairs + the pad-row boundary pairs
                    pr0 = (N_SLABS - 1) * ROWS_PER_SLAB
                    nc.gpsimd.tensor_tensor(
                        row_view(w2p, pr0, ROWS_PER_SLAB - 1),
                        row_view(w2, pr0, ROWS_PER_SLAB - 1),
                        row_view(w2, pr0 + 1, ROWS_PER_SLAB - 1),
                        op=ALU.add,
                    )
                    # rows -4..-1 (lead pad rows -4..-2 pair into data row 0)
                    nc.gpsimd.tensor_tensor(
                        row_view(w2p, -4, 4),
                        row_view(w2, -4, 4),
                        row_view(w2, -3, 4),
                        op=ALU.add,
                    )
                    # trailing: row H-1 pairs with pad row H (zero) etc
                    nc.gpsimd.tensor_tensor(
                        row_view(w2p, H - 1, 3),
                        row_view(w2, H - 1, 3),
                        row_view(w2, H, 3),
                        op=ALU.add,
                    )

                # ---------- stage 2: PE folds over H-shifts ----------
                for it in range(N_OUT_TILES):
                    i0 = it * OUT_TILE_ROWS
                    acc = ppool.tile([COUT, OUT_TILE_FREE], F32, tag="acc")

                    def rhs(buf, trow):
                        base = (i0 + trow) * PW2
                        return buf[:, base : base + OUT_TILE_ROWS * PW2].rearrange(
                            "c (r q) -> c r q", q=PW2
                        )[:, :, 3:115]

                    first = True
                    for t in range(1, 8):
                        nc.tensor.matmul(
                            acc[:],
                            pw[:, (t - 1) * COUT : t * COUT],
                            rhs(w1, t),
                            start=first,
                            stop=False,
                        )
                        first = False
                    if USE_PAIR_SUMS:
                        # box7(w2) = w2p[1] + w2p[3] + w2p[5] + w2[7]
                        for t in (1, 3, 5):
                            nc.tensor.matmul(
                                acc[:], bw[:], rhs(w2p, t), start=False, stop=False
                            )
                        nc.tensor.matmul(
                            acc[:], bw[:], rhs(w2, 7), start=False, stop=True
                        )
                    else:
                        for t in range(1, 8):
                            nc.tensor.matmul(
                                acc[:], bw[:], rhs(w2, t), start=False, stop=(t == 7)
                            )
                    ot = opool.tile([COUT, OUT_TILE_FREE], F32, tag="ot")
                    nc.scalar.activation(
                        ot[:], acc[:], AF.Identity, bias=bias_sb[:], scale=1.0
                    )
                    nc.sync.dma_start(
                        t_out[b, :, i0 : i0 + OUT_TILE_ROWS, :].rearrange(
                            "o r j -> o (r j)"
                        ),
                        ot[:],
                    )

    nc.compile()
    return nc


def make_in_maps(x, abc, bias):
    x = np.ascontiguousarray(x, dtype=np.float32)
    abc = np.asarray(abc, dtype=np.float32)
    bias = np.asarray(bias, dtype=np.float32)
    A, Bm, Cc = abc[0:128], abc[128:256], abc[256:384]
    pw = np.stack([(t - 4.0) * A + Cc for t in range(1, 8)]).astype(np.float32)
    in_maps = []
    for c in range(N_CORES):
        in_maps.append(
            {
                "xs": x[c * B_PER : (c + 1) * B_PER],
                "pw": pw,
                "bw": np.ascontiguousarray(Bm),
                "bias": np.ascontiguousarray(bias.reshape(COUT, 1)),
            }
        )
    return in_maps, N_CORES


def kernel(x: np.ndarray, abc: np.ndarray, bias: np.ndarray) -> np.ndarray:
    if "nc" not in _CACHE:
        _CACHE["nc"] = _build()
    nc = _CACHE["nc"]

    in_maps, _ = make_in_maps(x, abc, bias)
    res = run_bass_kernel_spmd(nc, in_maps, list(range(N_CORES)))
    out = np.concatenate([res.results[c]["out"] for c in range(N_CORES)], axis=0)
    return out.astype(np.float32)


if __name__ == "__main__":
    rng = np.random.default_rng(0)
    x = rng.standard_normal((16, 128, 112, 112), dtype=np.float32)
    abc = (rng.standard_normal((384, 128)) * 0.05).astype(np.float32)
    bias = (rng.standard_normal((128,)) * 0.05).astype(np.float32)
    out = kernel(x=x, abc=abc, bias=bias)
    print(out.shape, out.dtype)
